# revision 13
# baseline (speedup 1.0000x reference)
"""Nearest-E8-lattice quantizer (CachedE8Quantizer) as a Bass/Tile kernel on 8 trn2 cores.

Input x: [8388608, 8] fp32. Output: nearest point of E8 = D8 u (D8 + 1/2).
Sharding: data-parallel over the points dim, 1/8 per core (no comms).

Per 8-vector, everything derives from ONE rounding r0 = RNE(x), d0 = x - r0:
  a = |d0|, ma = max a, na = min a, sa = sum a,
  p0 = parity(sum r0), p1 = parity(sum r0 + #(d0>=0))
  D0 - D1 = p0*(1 - 2*ma) + sa - 2 - 2*p1*na   (squared-dist sums cancel)
  c = (D0 <= D1) -> branch 0 (D8), else branch 1 (D8 + 1/2)
  flip coord: branch0 at argmax a (dir sgn d0), branch1 at argmin a (dir -sgn d0)
  y = r0 + sigma*wpre, sigma = 2*(d0>=0)-1, wpre = c ? oh : 0.5-oh,
      oh = onehot(a == (c ? ma : na), gated off unless parity is odd... see mgq)
  All of r0, oh, wpre, y are bf16-exact (half-integers); y returned bf16,
  converted to fp32 on the host (exact).

Engines: Pool (GPSIMD) is intentionally unused — measured far slower than the
legacy cost model on TRN2. DVE does rounds/reduces/logic, ACT does |d| and
parity squares and PSUM evacuation, PE assembles y = I*r0 + 2I*z3 - I*wpre.
"""

import numpy as np

from concourse import bacc
import concourse.mybir as mybir
from concourse.alu_op_type import AluOpType as op
from concourse.tile import TileContext

N_POINTS = 8388608
N_CORES = 8
SHARD = N_POINTS // N_CORES  # 1048576 points per core

MAGIC = 12582912.0  # 1.5 * 2**23: (x + MAGIC) - MAGIC == round-half-even(x)
F32 = mybir.dt.float32
BF16 = mybir.dt.bfloat16
F16 = mybir.dt.float16
U32 = mybir.dt.uint32
X = mybir.AxisListType.X

TF = 256
PE_Y = True  # assemble y on PE (else DVE stt+add)


def _emit_tile(nc, pools, xd, yd, t, tf):
    P = 128
    pts = P * tf
    FE = tf * 8
    T = tf
    stream, work, small, pe = pools

    s = t * pts
    x_rows = xd[s : s + pts, :].rearrange("(p f) c -> p (f c)", p=P)
    y_rows = yd[s : s + pts, :].rearrange("(p f) c -> p (f c)", p=P)

    xt = stream.tile([P, FE], F32, tag="xt")
    nc.sync.dma_start(out=xt[:], in_=x_rows)

    # rr = [r0 | npos] bf16 (both exact: small ints / {0,1})
    rr = work.tile([P, 2 * FE], BF16, tag="rr")
    r0b = rr[:, :FE]
    nposb = rr[:, FE:]
    nc.vector.tensor_scalar(r0b, xt[:], MAGIC, MAGIC, op0=op.add, op1=op.subtract)

    dd = work.tile([P, FE], F32, tag="dd")
    nc.vector.tensor_tensor(dd[:], xt[:], r0b, op.subtract)  # exact
    nc.vector.tensor_scalar(nposb, dd[:], 0.0, None, op0=op.is_ge)
    aa = work.tile([P, FE], F32, tag="aa")
    nc.scalar.activation(aa[:], dd[:], mybir.ActivationFunctionType.Abs)

    rr3 = rr[:].rearrange("p (t c) -> p t c", c=8)  # [P, 2T, 8]
    aa3 = aa[:].rearrange("p (t c) -> p t c", c=8)  # [P, T, 8]

    # group-stage arena
    ar = small.tile([P, 10 * T], F32, tag="ar")
    sboth = ar[:, 0 * T : 2 * T]   # [s0 | s1]
    uu = ar[:, 2 * T : 4 * T]      # parity scratch (reused: e2, cf)
    pp = ar[:, 4 * T : 6 * T]      # [p0 | p1]
    ma = ar[:, 6 * T : 7 * T]
    na = ar[:, 7 * T : 8 * T]
    sa = ar[:, 8 * T : 9 * T]
    e1 = ar[:, 9 * T : 10 * T]     # reused: mgq
    sm2 = small.tile([P, 2 * T], F32, tag="sm2")
    msel = sm2[:, :T]
    psel = sm2[:, T:]
    ssh = small.tile([P, 2 * T], F16, tag="ssh")

    # segmented reduces (DVE-only capability)
    with nc.allow_low_precision(reason="sums of small ints, exact in f16"):
        nc.vector.tensor_reduce(ssh[:], rr3, axis=X, op=op.add)
    nc.vector.tensor_reduce(ma, aa3, axis=X, op=op.max)
    nc.vector.tensor_reduce(na, aa3, axis=X, op=op.min)
    nc.vector.tensor_reduce(sa, aa3, axis=X, op=op.add)

    # ---- group stage ----
    nc.vector.tensor_copy(sboth[:, :T], ssh[:, :T])
    nc.vector.tensor_tensor(sboth[:, T:], ssh[:, :T], ssh[:, T:], op.add)
    # parity of [s0|s1]: p = (2*round(s/2) - s)^2 in {0,1}
    nc.vector.tensor_scalar(uu, sboth, 0.5, MAGIC, op0=op.mult, op1=op.add)
    nc.vector.tensor_scalar(uu, uu, MAGIC, 2.0, op0=op.subtract, op1=op.mult)
    nc.vector.tensor_tensor(uu, uu, sboth, op.subtract)
    nc.scalar.square(pp, uu)
    p0 = pp[:, :T]
    p1 = pp[:, T:]
    # e1 = p0*(1-2ma) + (sa-2) ; e2 = 2*na*p1 ; c = e1 <= e2
    nc.vector.tensor_scalar(e1, ma, -2.0, 1.0, op0=op.mult, op1=op.add)
    nc.vector.tensor_tensor(e1, e1, p0, op.mult)
    nc.vector.scalar_tensor_tensor(e1, sa, 2.0, e1, op0=op.subtract, op1=op.add)
    e2 = uu[:, :T]
    nc.vector.scalar_tensor_tensor(e2, na, 2.0, p1, op0=op.mult, op1=op.mult)
    cf = uu[:, T:]
    nc.vector.tensor_tensor(cf, e1, e2, op.is_le)
    # msel = c ? ma : na ; psel = c ? p0 : p1
    cfu = cf.bitcast(U32)
    nc.vector.tensor_copy(msel, na)
    nc.vector.copy_predicated(msel, cfu, ma)
    nc.vector.tensor_copy(psel, p1)
    nc.vector.copy_predicated(psel, cfu, p0)
    # mgq = psel*msel + (psel-1): the active-branch max/min if its parity is
    # odd, else -1 (matches no |d0| value -> no flip)
    mgq = e1
    nc.vector.tensor_tensor(mgq, psel, msel, op.mult)
    nc.vector.scalar_tensor_tensor(mgq, psel, 1.0, mgq, op0=op.subtract, op1=op.add)

    # ---- elementwise finish ----
    ohb = work.tile([P, FE], BF16, tag="ohb")
    ohb3 = ohb[:].rearrange("p (t c) -> p t c", c=8)
    mgq_b = mgq.unsqueeze(2).broadcast_to([P, T, 8])
    nc.vector.tensor_tensor(ohb3, aa3, mgq_b, op.is_equal)
    # wpre = c ? oh : 0.5 - oh
    wpre = work.tile([P, FE], BF16, tag="wpre")
    wpre3 = wpre[:].rearrange("p (t c) -> p t c", c=8)
    nc.vector.tensor_scalar(wpre[:], ohb[:], -1.0, 0.5, op0=op.mult, op1=op.add)
    cI_b = cfu.unsqueeze(2).broadcast_to([P, T, 8])
    nc.vector.copy_predicated(wpre3, cI_b, ohb3)
    # z3 = wpre * npos (into npos slot); sigma*wpre = 2*z3 - wpre
    nc.vector.tensor_tensor(nposb, wpre[:], nposb, op.mult)
    ybb = stream.tile([P, FE], BF16, tag="ybb")
    if PE_Y:
        psum_pool, idb, nidb, id2b = pe
        yp = psum_pool.tile([P, FE], F32, tag="yp")
        NCH = 512
        for c0 in range(0, FE, NCH):
            sl = slice(c0, c0 + NCH)
            nc.tensor.matmul(yp[:, sl], id2b[:], rr[:, FE + c0 : FE + c0 + NCH], start=True, stop=False)
            nc.tensor.matmul(yp[:, sl], nidb[:], wpre[:, sl], start=False, stop=False)
            nc.tensor.matmul(yp[:, sl], idb[:], rr[:, c0 : c0 + NCH], start=False, stop=True)
        nc.scalar.copy(ybb[:], yp[:])
    else:
        nc.vector.scalar_tensor_tensor(wpre[:], nposb, 2.0, wpre[:], op0=op.mult, op1=op.subtract)
        nc.vector.tensor_tensor(ybb[:], r0b, wpre[:], op.add)
    nc.sync.dma_start(out=y_rows, in_=ybb[:])


def build_nc(shard=SHARD, tf=None, reps=1):
    if tf is None:
        tf = TF
    P = 128
    pts = P * tf
    assert shard % pts == 0
    ntiles = shard // pts

    nc = bacc.Bacc("TRN2", target_bir_lowering=False, debug=False, num_devices=N_CORES)
    xd = nc.declare_dram_parameter("x", [shard, 8], F32, isOutput=False)
    yd = nc.declare_dram_parameter("y", [shard, 8], BF16, isOutput=True)

    from concourse.masks import make_identity

    with TileContext(nc) as tc:
        with (
            tc.tile_pool(name="stream", bufs=2) as stream,
            tc.tile_pool(name="work", bufs=2) as work,
            tc.tile_pool(name="small", bufs=2) as small,
            tc.tile_pool(name="const", bufs=1) as cpool,
            tc.tile_pool(name="psum", bufs=2, space="PSUM") as psum_pool,
        ):
            pe = None
            if PE_Y:
                idb = cpool.tile([P, P], BF16, tag="idb")
                nidb = cpool.tile([P, P], BF16, tag="nidb")
                id2b = cpool.tile([P, P], BF16, tag="id2b")
                make_identity(nc, idb[:])
                CP = mybir.ActivationFunctionType.Copy
                nc.scalar.activation(nidb[:], idb[:], CP, scale=-1.0)
                nc.scalar.activation(id2b[:], idb[:], CP, scale=2.0)
                pe = (psum_pool, idb, nidb, id2b)
            for _ in range(reps):
                for t in range(ntiles):
                    _emit_tile(nc, (stream, work, small, pe), xd, yd, t, tf)
    nc.finalize()
    return nc


_BUILD_CACHE = {}
_RUNNER_CACHE = {}


def _get_runner(shard, tf):
    key = (shard, tf)
    if key not in _RUNNER_CACHE:
        import jax
        import jax.numpy as jnp
        from jax.experimental.shard_map import shard_map
        from jax.sharding import Mesh, NamedSharding, PartitionSpec
        from concourse.bass2jax import (
            _bass_exec_p,
            install_neuronx_cc_hook,
            partition_id_tensor,
        )

        install_neuronx_cc_hook()
        if key not in _BUILD_CACHE:
            _BUILD_CACHE[key] = build_nc(shard, tf)
        nc = _BUILD_CACHE[key]

        partition_name = (
            nc.partition_id_tensor.name if nc.partition_id_tensor else None
        )
        in_names, out_names, out_avals = [], [], []
        for alloc in nc.m.functions[0].allocations:
            if not isinstance(alloc, mybir.MemoryLocationSet):
                continue
            name = alloc.memorylocations[0].name
            if alloc.kind == "ExternalInput":
                if name != partition_name:
                    in_names.append(name)
            elif alloc.kind == "ExternalOutput":
                out_names.append(name)
                out_avals.append(
                    jax.core.ShapedArray(
                        tuple(alloc.tensor_shape), mybir.dt.np(alloc.dtype)
                    )
                )
        n_params = len(in_names)
        all_in = list(in_names) + list(out_names)
        if partition_name is not None:
            all_in.append(partition_name)

        def _body(*args):
            operands = list(args)
            if partition_name is not None:
                operands.append(partition_id_tensor())
            outs = _bass_exec_p.bind(
                *operands,
                out_avals=tuple(out_avals),
                in_names=tuple(all_in),
                out_names=tuple(out_names),
                lowering_input_output_aliases=(),
                sim_require_finite=True,
                sim_require_nnan=True,
                nc=nc,
            )
            return tuple(outs)

        devices = jax.devices()[:N_CORES]
        mesh = Mesh(np.asarray(devices), ("core",))
        spec = PartitionSpec("core")
        sharding = NamedSharding(mesh, spec)
        in_specs = (spec,) * (n_params + len(out_names))
        out_specs = (spec,) * len(out_names)
        fn = jax.jit(
            shard_map(
                _body, mesh=mesh, in_specs=in_specs, out_specs=out_specs, check_rep=False
            ),
            donate_argnums=tuple(range(n_params, n_params + len(out_names))),
            keep_unused=True,
        )
        zero_fns = []
        for aval in out_avals:
            shape = (N_CORES * aval.shape[0],) + tuple(aval.shape[1:])
            zero_fns.append(
                jax.jit(
                    lambda shape=shape, dtype=aval.dtype: jnp.zeros(shape, dtype),
                    out_shardings=sharding,
                )
            )
        _RUNNER_CACHE[key] = (fn, zero_fns, sharding)
    return _RUNNER_CACHE[key]


def kernel(x: np.ndarray) -> np.ndarray:
    import jax

    x = np.ascontiguousarray(x, dtype=np.float32)
    n = x.shape[0]
    shard = n // N_CORES
    tf = TF
    while shard % (128 * tf) != 0:
        tf //= 2
    fn, zero_fns, sharding = _get_runner(shard, tf)
    xdev = jax.device_put(x, sharding)
    zeros = [zf() for zf in zero_fns]
    (ybf,) = fn(xdev, *zeros)
    return np.asarray(ybf).astype(np.float32)
